# revision 23
# baseline (speedup 1.0000x reference)
"""Trainium2 Bass kernel for AttnDecoderRNN step (Bahdanau attention + GRU + vocab projection).

Sharding: batch-parallel (B=64 -> 8/core) for attention+GRU, vocab-parallel
(V=32000 -> 4000/core) for the output projection / log_softmax, with two tiny
device AllGathers in between (hstack features, then softmax stats).

kernel(**inputs) takes the full unsharded inputs and returns
(log_softmax_output [64, 32000] f32, h_new [1, 64, 512] f32).

Notes:
- Big matmuls run in fp32r (TF32) at full PE rate; fp32r-feeding tensors are
  declared float32r end-to-end and pre-rounded on the host (walrus birverifier
  requires every writer of an fp32r-matmul input to produce fp32r).
- Partition broadcasts are done with K=1 matmuls against a ones vector
  (gpsimd extended ops like partition_broadcast live in non-default Q7
  libraries and crash the runtime without a library load).
- Bias adds are folded into the matmul accumulation groups as K=1 rank-1
  updates (ones^T @ bias_row).
"""

import numpy as np

import concourse.bass as bass
import concourse.mybir as mybir
import concourse.tile as tile
from concourse import bacc
from concourse.bass import ts
from concourse.bass_utils import run_bass_kernel_spmd

# Problem shapes (hardcoded per contest contract)
B, S, H, E, V = 64, 2048, 512, 512, 32000
NCORES = 8
BC = B // NCORES          # 8 batch rows per core
F32 = mybir.dt.float32
F32R = mybir.dt.float32r  # fp32 bits, TF32 matmul mode (1 cyc/row when N>=256)


def build(nc, S_=S, VC=V // NCORES, dt_mm=F32R, stop_after='all'):
    """Emit the SPMD per-core program. Parameterized in S_ and VC for small-scale sim."""
    HC = H // 128             # 4 h-chunks
    KC = H // 128             # 4 k-chunks (attention energy dim = H)
    SCH = min(512, S_)        # s-chunk (moving N / psum bank)
    NSC = S_ // SCH           # s-chunks per b
    VCH = VC // 8             # vocab chunk per psum bank (500 at full size)
    GC = (H + H) // 128       # hstack feature chunks = 1024/128 = 8
    XKC = (H + E) // 128      # GRU input-feature chunks = 8
    EC = E // 128             # emb chunks = 4

    # ---------------- I/O ----------------
    d_encT = nc.dram_tensor("encTp", [BC, 128, HC, S_], dt_mm, kind="ExternalInput")
    d_attnWT = nc.dram_tensor("attn_WTp", [128, 2 * HC, H], dt_mm, kind="ExternalInput")
    d_attnbT = nc.dram_tensor("attn_bTp", [128, KC], F32, kind="ExternalInput")
    d_vT = nc.dram_tensor("vTp", [128, KC], dt_mm, kind="ExternalInput")
    d_hidT = nc.dram_tensor("hidTp", [128, HC, BC], dt_mm, kind="ExternalInput")
    d_embT = nc.dram_tensor("embTp", [128, EC, BC], dt_mm, kind="ExternalInput")
    d_hidN = nc.dram_tensor("hid_nat", [BC, H], F32, kind="ExternalInput")
    d_WihT = nc.dram_tensor("W_ihTp", [128, XKC, 3 * H], dt_mm, kind="ExternalInput")
    d_WhhT = nc.dram_tensor("W_hhTp", [128, HC, 3 * H], dt_mm, kind="ExternalInput")
    d_bih = nc.dram_tensor("b_ihr", [1, 3 * H], dt_mm, kind="ExternalInput")
    d_bhh = nc.dram_tensor("b_hhr", [1, 3 * H], dt_mm, kind="ExternalInput")
    d_outWT = nc.dram_tensor("out_WTp", [128, GC, VC], dt_mm, kind="ExternalInput")
    d_outb = nc.dram_tensor("out_bs", [1, VC], dt_mm, kind="ExternalInput")
    d_eye = nc.dram_tensor("eye8", [BC, BC], F32, kind="ExternalInput")
    d_ones = nc.dram_tensor("ones1", [1, 128], dt_mm, kind="ExternalInput")

    d_out = nc.dram_tensor("out_logits", [B, VC], F32, kind="ExternalOutput")
    d_hnew = nc.dram_tensor("h_new_out", [BC, H], F32, kind="ExternalOutput")

    with tile.TileContext(nc) as tc:
        with (
            tc.tile_pool(name="const", bufs=1) as cpool,
            tc.tile_pool(name="dram", bufs=1, space="DRAM") as dpool,
        ):
            # ---- resident constants ----
            attnWT = cpool.tile([128, 2 * HC, H], dt_mm)
            nc.sync.dma_start(attnWT[:], d_attnWT[:])
            attnbT = cpool.tile([128, KC], F32)
            nc.sync.dma_start(attnbT[:], d_attnbT[:])
            vT = cpool.tile([128, KC], dt_mm)
            nc.sync.dma_start(vT[:], d_vT[:])
            hidT = cpool.tile([128, HC, BC], dt_mm)
            nc.sync.dma_start(hidT[:], d_hidT[:])
            embT = cpool.tile([128, EC, BC], dt_mm)
            nc.sync.dma_start(embT[:], d_embT[:])
            hidN = cpool.tile([BC, H], F32)
            nc.sync.dma_start(hidN[:], d_hidN[:])
            eye8 = cpool.tile([BC, BC], F32)
            nc.sync.dma_start(eye8[:], d_eye[:])
            bih = cpool.tile([1, 3 * H], dt_mm)
            nc.sync.dma_start(bih[:], d_bih[:])
            bhh = cpool.tile([1, 3 * H], dt_mm)
            nc.sync.dma_start(bhh[:], d_bhh[:])
            ones1 = cpool.tile([1, 128], dt_mm)
            nc.sync.dma_start(ones1[:], d_ones[:])

            wih_t = cpool.tile([128, XKC, 3 * H], dt_mm)
            whh_t = cpool.tile([128, HC, 3 * H], dt_mm)
            for kc in range(HC):
                nc.sync.dma_start(whh_t[:, kc, :], d_WhhT[:, kc, :])
            for kc in range(XKC):
                nc.sync.dma_start(wih_t[:, kc, :], d_WihT[:, kc, :])
            ctxTr = cpool.tile([128, HC, BC], F32)     # unnormalized context^T
            ctxTs = cpool.tile([128, HC, BC], dt_mm)   # normalized, rounded (matmul lhsT)
            ctxTf = cpool.tile([128, HC, BC], F32)     # normalized, f32 staging
            l8row = cpool.tile([1, BC], F32)           # sum(exp(scores)) per b
            hstackT = cpool.tile([128, GC, BC], dt_mm)
            hsT_all = cpool.tile([128, GC, NCORES, BC], dt_mm)

            # ---- hbT = Wa @ hidT + attn_b   ([k, b] per k-chunk) ----
            hbT = cpool.tile([128, KC, BC], F32)
            with tc.tile_pool(name="hb_psum", bufs=2, space="PSUM") as hbps:
                for kc in range(KC):
                    ps_hb = hbps.tile([128, BC], F32, tag="ps_hb", name=f"ps_hb{kc}")
                    for hc in range(HC):
                        nc.tensor.matmul(
                            ps_hb[:],
                            attnWT[:, hc, ts(kc, 128)],
                            hidT[:, hc, :],
                            start=(hc == 0),
                            stop=(hc == HC - 1),
                        )
                    nc.scalar.activation(
                        hbT[:, kc, :], ps_hb[:],
                        mybir.ActivationFunctionType.Identity,
                        bias=attnbT[:, kc : kc + 1],
                    )

            # ---- GRU parts that don't need context: gh = hid@W_hh^T + b_hh,
            # gi_emb = emb@W_ih[:,:E]^T + b_ih  (runs early, also warms the PE) ----
            gh_sb = cpool.tile([BC, 3 * H], F32)
            gie_sb = cpool.tile([BC, 3 * H], F32)
            with tc.tile_pool(name="g0_psum", bufs=1, space="PSUM") as g0ps:
                ps_gh0 = g0ps.tile([BC, 3 * H], F32)
                for kc in range(HC):
                    for j in range(3 * H // 512):
                        nc.tensor.matmul(
                            ps_gh0[:, ts(j, 512)], hidT[:, kc, :], whh_t[:, kc, ts(j, 512)],
                            start=(kc == 0), stop=False,
                        )
                for j in range(3 * H // 512):
                    nc.tensor.matmul(
                        ps_gh0[:, ts(j, 512)],
                        ones1[0:1, 0:BC], bhh[0:1, ts(j, 512)],
                        start=False, stop=True,
                    )
                nc.scalar.copy(gh_sb[:], ps_gh0[:])
                ps_gi0 = g0ps.tile([BC, 3 * H], F32)
                for kc in range(EC):
                    for j in range(3 * H // 512):
                        nc.tensor.matmul(
                            ps_gi0[:, ts(j, 512)], embT[:, kc, :], wih_t[:, kc, ts(j, 512)],
                            start=(kc == 0), stop=False,
                        )
                for j in range(3 * H // 512):
                    nc.tensor.matmul(
                        ps_gi0[:, ts(j, 512)],
                        ones1[0:1, 0:BC], bih[0:1, ts(j, 512)],
                        start=False, stop=True,
                    )
                nc.scalar.copy(gie_sb[:], ps_gi0[:])

            LEVELS = {'hb': 0, 'energy': 1, 'vdot': 2, 'softmax': 3, 'p1': 4,
                      'gru': 5, 'cc1': 6, 'all': 9}
            LVL = LEVELS[stop_after]
            if LVL == 0:
                nc.sync.dma_start(d_hnew[0:8, 0:32], hbT[0:8, :, :])
                return nc

            # ---- phase 1: attention, software-pipelined over batch rows ----
            # stage A(b): energy + tanh + vdot (PE/ACT heavy)
            # stage B(b): softmax + p-broadcast + context (ACT/DVE heavy)
            # B(b-1) is emitted after A(b) so the PE stays dense through the
            # softmax tail (HAM stays warm).
            with (
                tc.tile_pool(name="p1", bufs=1) as p1,
                tc.tile_pool(name="p1_psum", bufs=1, space="PSUM") as p1ps,
            ):
                state = {}

                def stage_a(b):
                    enc_b = [
                        p1.tile([128, S_], dt_mm, tag="enc", bufs=8, name=f"enc_{b}_{hc}")
                        for hc in range(HC)
                    ]
                    for hc in range(HC):
                        nc.sync.dma_start(enc_b[hc][:], d_encT[b, :, hc, :])
                    ps_scs = []
                    for sc in range(NSC):
                        ps_sc = p1ps.tile([1, SCH], F32, tag="ps_sc", bufs=NSC,
                                          name=f"ps_sc{b}_{sc}")
                        ps_scs.append(ps_sc)
                        for kc in range(KC):
                            ps_e = p1ps.tile([128, SCH], F32, tag="ps_e", bufs=2,
                                             name=f"ps_e{b}_{sc}_{kc}")
                            for hc in range(HC):
                                nc.tensor.matmul(
                                    ps_e[:],
                                    attnWT[:, HC + hc, ts(kc, 128)],
                                    enc_b[hc][:, ts(sc, SCH)],
                                    start=(hc == 0),
                                    stop=(hc == HC - 1),
                                )
                            tanh_t = p1.tile([128, SCH], dt_mm, tag="tanh", bufs=2,
                                             name=f"tanh{b}_{sc}_{kc}")
                            nc.scalar.activation(
                                tanh_t[:], ps_e[:],
                                mybir.ActivationFunctionType.Tanh,
                                bias=hbT[:, kc, b : b + 1],
                            )
                            nc.tensor.matmul(
                                ps_sc[0:1, :],
                                vT[:, kc : kc + 1],
                                tanh_t[:],
                                start=(kc == 0),
                                stop=(kc == KC - 1),
                            )
                    state[b] = (enc_b, ps_scs)

                def stage_b(b):
                    enc_b, ps_scs = state.pop(b)
                    mx4 = p1.tile([1, NSC], F32, tag="mx4", bufs=2, name=f"mx4_{b}")
                    for sc in range(NSC):
                        nc.vector.reduce_max(mx4[0:1, sc : sc + 1], ps_scs[sc][:],
                                             axis=mybir.AxisListType.X)
                    mx = p1.tile([1, 1], F32, tag="mx", bufs=2, name=f"mx{b}")
                    nc.vector.reduce_max(mx[:], mx4[:], axis=mybir.AxisListType.X)
                    nmx = p1.tile([1, 1], F32, tag="nmx", bufs=2, name=f"nmx{b}")
                    nc.scalar.mul(nmx[:], mx[:], -1.0)
                    p_exp = p1.tile([1, S_], dt_mm, tag="p_exp", bufs=1, name=f"p_exp{b}")
                    l8part = p1.tile([1, NSC], F32, tag="l8part", bufs=2, name=f"l8p{b}")
                    for sc in range(NSC):
                        nc.scalar.activation(
                            p_exp[0:1, ts(sc, SCH)], ps_scs[sc][:],
                            mybir.ActivationFunctionType.Exp,
                            bias=nmx[:],
                            accum_out=l8part[0:1, sc : sc + 1],
                        )
                    nc.vector.reduce_sum(l8row[0:1, b : b + 1], l8part[:],
                                         axis=mybir.AxisListType.X)
                    part4 = p1.tile([128, HC, NSC], F32, tag="part4", bufs=2,
                                    name=f"part4_{b}")
                    for sc in range(NSC):
                        p_bc = p1ps.tile([128, SCH], F32, tag="p_bc", bufs=2,
                                         name=f"p_bc{b}_{sc}")
                        nc.tensor.matmul(
                            p_bc[:], ones1[:], p_exp[0:1, ts(sc, SCH)],
                            start=True, stop=True,
                        )
                        for hc in range(HC):
                            prod = p1.tile([128, SCH], F32, tag="prod", bufs=2,
                                           name=f"prod{b}_{hc}_{sc}")
                            # fused multiply + row-sum in one DVE pass:
                            # out = (enc * 1.0) * p ; accum_out = sum(out)
                            nc.vector.scalar_tensor_tensor(
                                out=prod[:],
                                in0=enc_b[hc][:, ts(sc, SCH)].bitcast(F32),
                                scalar=1.0,
                                in1=p_bc[:],
                                op0=mybir.AluOpType.mult,
                                op1=mybir.AluOpType.mult,
                                accum_out=part4[:, hc, sc : sc + 1],
                            )
                    for hc in range(HC):
                        nc.vector.reduce_sum(
                            ctxTr[:, hc, b : b + 1], part4[:, hc, :],
                            axis=mybir.AxisListType.X,
                        )

                for b in range(BC + 1):
                    if b < BC:
                        stage_a(b)
                    if b > 0:
                        stage_b(b - 1)

            # ---- GRU (natural [b, feature] layout; b on partitions 0..7) ----
            with (
                tc.tile_pool(name="gru", bufs=1) as gp,
                tc.tile_pool(name="gru_psum", bufs=1, space="PSUM") as gps,
            ):
                # normalize context by 1/sum(exp): rl broadcast via K=1 matmul
                rl = gp.tile([1, BC], F32)
                nc.vector.reciprocal(rl[:], l8row[:])
                rl_bc = gps.tile([128, BC], F32, tag="rl_bc")
                nc.tensor.matmul(rl_bc[:], ones1[:].bitcast(F32), rl[:],
                                 start=True, stop=True)
                for hc in range(HC):
                    nc.vector.tensor_tensor(
                        ctxTf[:, hc, :], ctxTr[:, hc, :], rl_bc[:],
                        mybir.AluOpType.mult,
                    )
                    nc.scalar.copy(ctxTs[:, hc, :], ctxTf[:, hc, :])

                ps_gi = gps.tile([BC, 3 * H], F32)
                for kc in range(EC, XKC):
                    for j in range(3 * H // 512):
                        nc.tensor.matmul(
                            ps_gi[:, ts(j, 512)], ctxTs[:, kc - EC, :],
                            wih_t[:, kc, ts(j, 512)],
                            start=(kc == EC), stop=(kc == XKC - 1),
                        )
                # gi = gie_sb + ps_gi ; gh already in gh_sb (biases included)
                gi_sb = gp.tile([BC, 3 * H], F32)
                nc.vector.tensor_tensor(gi_sb[:], gie_sb[:], ps_gi[:],
                                        mybir.AluOpType.add)
                g_rz = gp.tile([BC, 2 * H], F32)
                nc.vector.tensor_tensor(g_rz[:], gi_sb[:, : 2 * H], gh_sb[:, : 2 * H],
                                        mybir.AluOpType.add)
                r_g = gp.tile([BC, H], F32)
                nc.scalar.activation(r_g[:], g_rz[:, :H],
                                     mybir.ActivationFunctionType.Sigmoid)
                z_g = gp.tile([BC, H], F32)
                nc.scalar.activation(z_g[:], g_rz[:, H:],
                                     mybir.ActivationFunctionType.Sigmoid)
                # n = tanh(gi_n + r * gh_n)
                u1 = gp.tile([BC, H], F32)
                nc.vector.tensor_tensor(u1[:], r_g[:], gh_sb[:, 2 * H :],
                                        mybir.AluOpType.mult)
                nc.vector.tensor_tensor(u1[:], u1[:], gi_sb[:, 2 * H :],
                                        mybir.AluOpType.add)
                n_g = gp.tile([BC, H], F32)
                nc.scalar.activation(n_g[:], u1[:], mybir.ActivationFunctionType.Tanh)
                # h_new = n + z * (hid - n)
                hmn = gp.tile([BC, H], F32)
                nc.vector.tensor_tensor(hmn[:], hidN[:], n_g[:], mybir.AluOpType.subtract)
                nc.vector.tensor_tensor(hmn[:], hmn[:], z_g[:], mybir.AluOpType.mult)
                h_new = gp.tile([BC, H], F32)
                nc.vector.tensor_tensor(h_new[:], n_g[:], hmn[:], mybir.AluOpType.add)
                nc.sync.dma_start(d_hnew[:], h_new[:])

                # hstackT = [h_new^T ; ctx^T]  (f32r)
                for hc in range(HC):
                    ps_t = gps.tile([128, BC], F32, tag="ps_t", bufs=1, name=f"ps_t{hc}")
                    nc.tensor.transpose(ps_t[:], h_new[:, ts(hc, 128)], eye8[:])
                    nc.scalar.copy(hstackT[:, hc, :], ps_t[:])
                    nc.scalar.copy(hstackT[:, HC + hc, :], ctxTf[:, hc, :])

            if stop_after == 'gru':
                return nc

            # ---- AllGather hstackT across cores ----
            cc_in = dpool.tile([128, GC, BC], dt_mm)
            nc.sync.dma_start(cc_in[:], hstackT[:])
            gathered = dpool.tile([NCORES, 128, GC, BC], dt_mm, addr_space="Shared")
            nc.gpsimd.collective_compute(
                "AllGather",
                mybir.AluOpType.bypass,
                replica_groups=[list(range(NCORES))],
                ins=[cc_in.opt()],
                outs=[gathered.opt()],
            )
            nc.sync.dma_start(
                hsT_all[:], gathered[:].rearrange("n p c b -> p c n b")
            )

            if stop_after == 'cc1':
                nc.sync.dma_start(d_out[0:64, 0:GC * BC], hsT_all[0:64, :, 0, :])
                return nc

            # ---- phase 2: logits for the local vocab slice + log_softmax ----
            with (
                tc.tile_pool(name="p2", bufs=1) as p2,
                tc.tile_pool(name="p2_psum", bufs=1, space="PSUM") as p2ps,
            ):
                outb = p2.tile([1, VC], dt_mm)
                nc.sync.dma_start(outb[:], d_outb[:])
                logits = p2.tile([B, VC], F32)
                lmax8 = p2.tile([B, 8], F32)
                lsum8 = p2.tile([B, 8], F32)
                ps_l = [p2ps.tile([B, VCH], F32, name=f"ps_l{j}") for j in range(8)]
                for kc in range(GC):
                    w_t = p2.tile([128, VC], dt_mm, tag="outw", bufs=2, name=f"outw{kc}")
                    nc.sync.dma_start(w_t[:], d_outWT[:, kc, :])
                    for j in range(8):
                        nc.tensor.matmul(
                            ps_l[j][:],
                            hsT_all[:, kc, :, :],
                            w_t[:, ts(j, VCH)],
                            start=(kc == 0),
                            stop=False,
                        )
                        if kc == GC - 1:
                            # out_b as K=1 rank-1 accumulation closes the bank;
                            # epilogue for this j overlaps remaining j's matmuls
                            nc.tensor.matmul(
                                ps_l[j][:],
                                ones1[0:1, 0:B], outb[0:1, ts(j, VCH)],
                                start=False, stop=True,
                            )
                            nc.scalar.copy(logits[:, ts(j, VCH)], ps_l[j][:])
                            nc.vector.reduce_max(lmax8[:, j : j + 1],
                                                 logits[:, ts(j, VCH)],
                                                 axis=mybir.AxisListType.X)
                lmax = p2.tile([B, 1], F32)
                nc.vector.reduce_max(lmax[:], lmax8[:], axis=mybir.AxisListType.X)
                nlmax = p2.tile([B, 1], F32)
                nc.scalar.mul(nlmax[:], lmax[:], -1.0)
                for j in range(8):
                    e_scr = p2.tile([B, VCH], F32, tag="escr", bufs=2, name=f"escr{j}")
                    nc.scalar.activation(
                        e_scr[:], logits[:, ts(j, VCH)],
                        mybir.ActivationFunctionType.Exp,
                        bias=nlmax[:],
                        accum_out=lsum8[:, j : j + 1],
                    )
                lsum = p2.tile([B, 1], F32)
                nc.vector.reduce_sum(lsum[:], lsum8[:], axis=mybir.AxisListType.X)

                # AllGather (lmax, lsum) pairs and combine
                st2 = p2.tile([B, 2], F32)
                nc.vector.tensor_copy(st2[:, 0:1], lmax[:])
                nc.vector.tensor_copy(st2[:, 1:2], lsum[:])
                cc2 = dpool.tile([B, 2], F32)
                nc.sync.dma_start(cc2[:], st2[:])
                g2d = dpool.tile([NCORES, B, 2], F32, addr_space="Shared")
                nc.gpsimd.collective_compute(
                    "AllGather",
                    mybir.AluOpType.bypass,
                    replica_groups=[list(range(NCORES))],
                    ins=[cc2.opt()],
                    outs=[g2d.opt()],
                )
                g2 = p2.tile([B, NCORES, 2], F32)
                nc.sync.dma_start(g2[:], g2d[:].rearrange("n p t -> p n t"))
                gmax = p2.tile([B, 1], F32)
                nc.vector.reduce_max(gmax[:], g2[:, :, 0], axis=mybir.AxisListType.X)
                ngmax = p2.tile([B, 1], F32)
                nc.scalar.mul(ngmax[:], gmax[:], -1.0)
                wexp = p2.tile([B, NCORES], F32)
                nc.scalar.activation(
                    wexp[:], g2[:, :, 0],
                    mybir.ActivationFunctionType.Exp,
                    bias=ngmax[:],
                )
                wsc = p2.tile([B, NCORES], F32)
                gsum = p2.tile([B, 1], F32)
                nc.vector.tensor_tensor(wsc[:], wexp[:], g2[:, :, 1],
                                        mybir.AluOpType.mult)
                nc.vector.reduce_sum(gsum[:], wsc[:], axis=mybir.AxisListType.X)
                lng = p2.tile([B, 1], F32)
                nc.scalar.activation(lng[:], gsum[:], mybir.ActivationFunctionType.Ln)
                corr = p2.tile([B, 1], F32)
                nc.vector.tensor_tensor(corr[:], gmax[:], lng[:], mybir.AluOpType.add)
                ncorr = p2.tile([B, 1], F32)
                nc.scalar.mul(ncorr[:], corr[:], -1.0)
                nc.vector.tensor_scalar_add(logits[:], logits[:], ncorr[:])
                nc.sync.dma_start(d_out[:], logits[:])

    return nc


# ------------------------------------------------------------------
# Host side
# ------------------------------------------------------------------

def tf32_round(x):
    """Round fp32 to TF32 (10-bit mantissa, RNE) — the PE's fp32r operand format."""
    u = np.ascontiguousarray(x, np.float32).view(np.uint32)
    r = (u + 0x0FFF + ((u >> 13) & 1)) & 0xFFFFE000
    return r.view(np.float32)


def _pack_chunks(mat):
    """[R, C] with R % 128 == 0 -> [128, R//128, C] chunk-major packing."""
    R, C = mat.shape
    return np.ascontiguousarray(mat.reshape(R // 128, 128, C).transpose(1, 0, 2))


def prepare_inputs(word_input, last_hidden, encoder_outputs, emb, attn_W, attn_b, v,
                   W_ih, W_hh, b_ih, b_hh, out_W, out_b):
    f = np.float32
    word = np.asarray(word_input).astype(np.int64)
    hid = np.asarray(last_hidden, f)[0]             # [B, H]
    enc = np.asarray(encoder_outputs, f)            # [S, B, H]
    emb = np.asarray(emb, f)
    attn_W = np.asarray(attn_W, f)
    attn_b = np.asarray(attn_b, f)
    v = np.asarray(v, f)
    W_ih = np.asarray(W_ih, f)
    W_hh = np.asarray(W_hh, f)
    b_ih = np.asarray(b_ih, f)
    b_hh = np.asarray(b_hh, f)
    out_W = np.asarray(out_W, f)
    out_b = np.asarray(out_b, f)
    S_ = enc.shape[0]
    V_ = out_W.shape[0]
    VC = V_ // NCORES

    # shared (replicated) tensors (pre-rounded to TF32 where they feed fp32r matmuls)
    attn_WTp = tf32_round(_pack_chunks(np.ascontiguousarray(attn_W.T)))  # [128, 8, 512]
    attn_bTp = np.ascontiguousarray(attn_b.reshape(4, 128).T)     # [128, 4]
    vTp = tf32_round(np.ascontiguousarray(v.reshape(4, 128).T))   # [128, 4]
    W_ihTp = tf32_round(_pack_chunks(np.ascontiguousarray(W_ih.T)))  # [128, 8, 1536]
    W_hhTp = tf32_round(_pack_chunks(np.ascontiguousarray(W_hh.T)))  # [128, 4, 1536]
    eye8 = np.eye(BC, dtype=f)
    ones1 = np.ones((1, 128), f)
    b_ihr = tf32_round(np.ascontiguousarray(b_ih.reshape(1, -1)))
    b_hhr = tf32_round(np.ascontiguousarray(b_hh.reshape(1, -1)))

    # enc transposed per batch: [B, 128, 4, S]
    encT = np.ascontiguousarray(enc.transpose(1, 2, 0))           # [B, H, S]
    encT = tf32_round(np.ascontiguousarray(
        encT.reshape(B, 4, 128, S_).transpose(0, 2, 1, 3)))       # [B, 128, 4, S]

    x_emb = emb[word]                                             # [B, E]
    out_WT = np.ascontiguousarray(out_W.T)                        # [2H, V]

    in_maps = []
    for c in range(NCORES):
        bs = slice(c * BC, (c + 1) * BC)
        vs = slice(c * VC, (c + 1) * VC)
        hid_c = np.ascontiguousarray(hid[bs])                     # [BC, H]
        hidTp = tf32_round(np.ascontiguousarray(
            hid_c.T.reshape(4, 128, BC).transpose(1, 0, 2)))      # [128, 4, BC]
        embTp = tf32_round(np.ascontiguousarray(
            x_emb[bs].T.reshape(4, 128, BC).transpose(1, 0, 2)))  # [128, 4, BC]
        out_WTp = tf32_round(_pack_chunks(np.ascontiguousarray(out_WT[:, vs])))  # [128, 8, VC]
        in_maps.append({
            "encTp": np.ascontiguousarray(encT[bs]),
            "attn_WTp": attn_WTp,
            "attn_bTp": attn_bTp,
            "vTp": vTp,
            "hidTp": hidTp,
            "embTp": embTp,
            "hid_nat": hid_c,
            "W_ihTp": W_ihTp,
            "W_hhTp": W_hhTp,
            "b_ihr": b_ihr,
            "b_hhr": b_hhr,
            "out_WTp": out_WTp,
            "out_bs": tf32_round(np.ascontiguousarray(out_b[vs].reshape(1, -1))),
            "eye8": eye8,
            "ones1": ones1,
        })
    return in_maps


_CACHE = {}


def get_nc(S_=S, VC=V // NCORES, stop_after='all'):
    key = (S_, VC, stop_after)
    if key not in _CACHE:
        nc = bacc.Bacc("TRN2", target_bir_lowering=False, debug=False,
                       enable_asserts=False, num_devices=NCORES)
        build(nc, S_=S_, VC=VC, stop_after=stop_after)
        nc.compile()
        _CACHE[key] = nc
    return _CACHE[key]


def run(in_maps, trace=False, stop_after='all', **kw):
    S_ = in_maps[0]["encTp"].shape[-1]
    VC = in_maps[0]["out_WTp"].shape[-1]
    nc = get_nc(S_, VC, stop_after)
    return run_bass_kernel_spmd(nc, in_maps, core_ids=list(range(NCORES)),
                                trace=trace, **kw)


def kernel(**inputs):
    in_maps = prepare_inputs(**inputs)
    res = run(in_maps)
    out = np.concatenate(
        [np.asarray(res.results[c]["out_logits"]) for c in range(NCORES)], axis=1)
    h_new = np.concatenate(
        [np.asarray(res.results[c]["h_new_out"]) for c in range(NCORES)], axis=0)
    return out, h_new[None]


# revision 24
# speedup vs baseline: 1.2022x; 1.2022x over previous
"""Trainium2 Bass kernel for AttnDecoderRNN step (Bahdanau attention + GRU + vocab projection).

Sharding: batch-parallel (B=64 -> 8/core) for attention+GRU, vocab-parallel
(V=32000 -> 4000/core) for the output projection / log_softmax, with two tiny
device AllGathers in between (hstack features, then softmax stats).

kernel(**inputs) takes the full unsharded inputs and returns
(log_softmax_output [64, 32000] f32, h_new [1, 64, 512] f32).

Notes:
- Big matmuls run in fp32r (TF32) at full PE rate; fp32r-feeding tensors are
  declared float32r end-to-end and pre-rounded on the host (walrus birverifier
  requires every writer of an fp32r-matmul input to produce fp32r).
- Partition broadcasts are done with K=1 matmuls against a ones vector
  (gpsimd extended ops like partition_broadcast live in non-default Q7
  libraries and crash the runtime without a library load).
- Bias adds are folded into the matmul accumulation groups as K=1 rank-1
  updates (ones^T @ bias_row).
"""

import numpy as np

import concourse.bass as bass
import concourse.mybir as mybir
import concourse.tile as tile
from concourse import bacc
from concourse.bass import ts
from concourse.bass_utils import run_bass_kernel_spmd

# Problem shapes (hardcoded per contest contract)
B, S, H, E, V = 64, 2048, 512, 512, 32000
NCORES = 8
BC = B // NCORES          # 8 batch rows per core
F32 = mybir.dt.float32
F32R = mybir.dt.float32r  # fp32 bits, TF32 matmul mode (1 cyc/row when N>=256)


def build(nc, S_=S, VC=V // NCORES, dt_mm=F32R, stop_after='all'):
    """Emit the SPMD per-core program. Parameterized in S_ and VC for small-scale sim."""
    HC = H // 128             # 4 h-chunks
    KC = H // 128             # 4 k-chunks (attention energy dim = H)
    SCH = min(512, S_)        # s-chunk (moving N / psum bank)
    NSC = S_ // SCH           # s-chunks per b
    VCH = VC // 8             # vocab chunk per psum bank (500 at full size)
    GC = (H + H) // 128       # hstack feature chunks = 1024/128 = 8
    XKC = (H + E) // 128      # GRU input-feature chunks = 8
    EC = E // 128             # emb chunks = 4

    # ---------------- I/O ----------------
    d_encT = nc.dram_tensor("encTp", [BC, 128, HC, S_], dt_mm, kind="ExternalInput")
    d_attnWT = nc.dram_tensor("attn_WTp", [128, 2 * HC, H], dt_mm, kind="ExternalInput")
    d_attnbT = nc.dram_tensor("attn_bTp", [128, KC], F32, kind="ExternalInput")
    d_vT = nc.dram_tensor("vTp", [128, KC], dt_mm, kind="ExternalInput")
    d_hidT = nc.dram_tensor("hidTp", [128, HC, BC], dt_mm, kind="ExternalInput")
    d_embT = nc.dram_tensor("embTp", [128, EC, BC], dt_mm, kind="ExternalInput")
    d_hidN = nc.dram_tensor("hid_nat", [BC, H], F32, kind="ExternalInput")
    d_WihT = nc.dram_tensor("W_ihTp", [128, XKC, 3 * H], dt_mm, kind="ExternalInput")
    d_WhhT = nc.dram_tensor("W_hhTp", [128, HC, 3 * H], dt_mm, kind="ExternalInput")
    d_bih = nc.dram_tensor("b_ihr", [1, 3 * H], dt_mm, kind="ExternalInput")
    d_bhh = nc.dram_tensor("b_hhr", [1, 3 * H], dt_mm, kind="ExternalInput")
    d_outWT = nc.dram_tensor("out_WTp", [128, GC, VC], dt_mm, kind="ExternalInput")
    d_outb = nc.dram_tensor("out_bs", [1, VC], dt_mm, kind="ExternalInput")
    d_eye = nc.dram_tensor("eye8", [BC, BC], F32, kind="ExternalInput")
    d_ones = nc.dram_tensor("ones1", [1, 128], dt_mm, kind="ExternalInput")

    d_out = nc.dram_tensor("out_logits", [B, VC], F32, kind="ExternalOutput")
    d_hnew = nc.dram_tensor("h_new_out", [BC, H], F32, kind="ExternalOutput")

    with tile.TileContext(nc) as tc:
        with (
            tc.tile_pool(name="const", bufs=1) as cpool,
            tc.tile_pool(name="dram", bufs=1, space="DRAM") as dpool,
        ):
            # ---- resident constants ----
            attnWT = cpool.tile([128, 2 * HC, H], dt_mm)
            nc.sync.dma_start(attnWT[:], d_attnWT[:])
            attnbT = cpool.tile([128, KC], F32)
            nc.sync.dma_start(attnbT[:], d_attnbT[:])
            vT = cpool.tile([128, KC], dt_mm)
            nc.sync.dma_start(vT[:], d_vT[:])
            hidT = cpool.tile([128, HC, BC], dt_mm)
            nc.sync.dma_start(hidT[:], d_hidT[:])
            embT = cpool.tile([128, EC, BC], dt_mm)
            nc.sync.dma_start(embT[:], d_embT[:])
            hidN = cpool.tile([BC, H], F32)
            nc.sync.dma_start(hidN[:], d_hidN[:])
            eye8 = cpool.tile([BC, BC], F32)
            nc.sync.dma_start(eye8[:], d_eye[:])
            bih = cpool.tile([1, 3 * H], dt_mm)
            nc.sync.dma_start(bih[:], d_bih[:])
            bhh = cpool.tile([1, 3 * H], dt_mm)
            nc.sync.dma_start(bhh[:], d_bhh[:])
            ones1 = cpool.tile([1, 128], dt_mm)
            nc.sync.dma_start(ones1[:], d_ones[:])

            wih_t = cpool.tile([128, XKC, 3 * H], dt_mm)   # loaded during phase 1
            whh_t = cpool.tile([128, HC, 3 * H], dt_mm)
            ctxTr = cpool.tile([128, HC, BC], F32)     # unnormalized context^T
            ctxTs = cpool.tile([128, HC, BC], dt_mm)   # normalized, rounded (matmul lhsT)
            ctxTf = cpool.tile([128, HC, BC], F32)     # normalized, f32 staging
            l8row = cpool.tile([1, BC], F32)           # sum(exp(scores)) per b
            hstackT = cpool.tile([128, GC, BC], dt_mm)
            hsT_all = cpool.tile([128, GC, NCORES, BC], dt_mm)

            # ---- hbT = Wa @ hidT + attn_b   ([k, b] per k-chunk) ----
            hbT = cpool.tile([128, KC, BC], F32)
            with tc.tile_pool(name="hb_psum", bufs=2, space="PSUM") as hbps:
                for kc in range(KC):
                    ps_hb = hbps.tile([128, BC], F32, tag="ps_hb", name=f"ps_hb{kc}")
                    for hc in range(HC):
                        nc.tensor.matmul(
                            ps_hb[:],
                            attnWT[:, hc, ts(kc, 128)],
                            hidT[:, hc, :],
                            start=(hc == 0),
                            stop=(hc == HC - 1),
                        )
                    nc.scalar.activation(
                        hbT[:, kc, :], ps_hb[:],
                        mybir.ActivationFunctionType.Identity,
                        bias=attnbT[:, kc : kc + 1],
                    )

            LEVELS = {'hb': 0, 'energy': 1, 'vdot': 2, 'softmax': 3, 'p1': 4,
                      'gru': 5, 'cc1': 6, 'all': 9}
            LVL = LEVELS[stop_after]
            if LVL == 0:
                nc.sync.dma_start(d_hnew[0:8, 0:32], hbT[0:8, :, :])
                return nc

            # ---- phase 1: attention, software-pipelined over batch rows ----
            # stage A(b): energy + tanh + vdot (PE/ACT heavy)
            # stage B(b): softmax + p-broadcast + context (ACT/DVE heavy)
            # B(b-1) is emitted after A(b) so the PE stays dense through the
            # softmax tail (HAM stays warm).
            with (
                tc.tile_pool(name="p1", bufs=1) as p1,
                tc.tile_pool(name="p1_psum", bufs=1, space="PSUM") as p1ps,
            ):
                state = {}

                def stage_a(b):
                    enc_b = [
                        p1.tile([128, S_], dt_mm, tag="enc", bufs=8, name=f"enc_{b}_{hc}")
                        for hc in range(HC)
                    ]
                    for hc in range(HC):
                        nc.sync.dma_start(enc_b[hc][:], d_encT[b, :, hc, :])
                    if b == 0:
                        # spread the GRU weight loads into phase 1 (DMA slack)
                        for kc in range(XKC):
                            nc.sync.dma_start(wih_t[:, kc, :], d_WihT[:, kc, :])
                        for kc in range(HC):
                            nc.sync.dma_start(whh_t[:, kc, :], d_WhhT[:, kc, :])
                    ps_scs = []
                    for sc in range(NSC):
                        ps_sc = p1ps.tile([1, SCH], F32, tag="ps_sc", bufs=NSC,
                                          name=f"ps_sc{b}_{sc}")
                        ps_scs.append(ps_sc)
                        for kc in range(KC):
                            ps_e = p1ps.tile([128, SCH], F32, tag="ps_e", bufs=2,
                                             name=f"ps_e{b}_{sc}_{kc}")
                            for hc in range(HC):
                                nc.tensor.matmul(
                                    ps_e[:],
                                    attnWT[:, HC + hc, ts(kc, 128)],
                                    enc_b[hc][:, ts(sc, SCH)],
                                    start=(hc == 0),
                                    stop=(hc == HC - 1),
                                )
                            tanh_t = p1.tile([128, SCH], dt_mm, tag="tanh", bufs=2,
                                             name=f"tanh{b}_{sc}_{kc}")
                            nc.scalar.activation(
                                tanh_t[:], ps_e[:],
                                mybir.ActivationFunctionType.Tanh,
                                bias=hbT[:, kc, b : b + 1],
                            )
                            nc.tensor.matmul(
                                ps_sc[0:1, :],
                                vT[:, kc : kc + 1],
                                tanh_t[:],
                                start=(kc == 0),
                                stop=(kc == KC - 1),
                            )
                    state[b] = (enc_b, ps_scs)

                def stage_b(b):
                    enc_b, ps_scs = state.pop(b)
                    mx4 = p1.tile([1, NSC], F32, tag="mx4", bufs=2, name=f"mx4_{b}")
                    for sc in range(NSC):
                        nc.vector.reduce_max(mx4[0:1, sc : sc + 1], ps_scs[sc][:],
                                             axis=mybir.AxisListType.X)
                    mx = p1.tile([1, 1], F32, tag="mx", bufs=2, name=f"mx{b}")
                    nc.vector.reduce_max(mx[:], mx4[:], axis=mybir.AxisListType.X)
                    nmx = p1.tile([1, 1], F32, tag="nmx", bufs=2, name=f"nmx{b}")
                    nc.scalar.mul(nmx[:], mx[:], -1.0)
                    p_exp = p1.tile([1, S_], dt_mm, tag="p_exp", bufs=1, name=f"p_exp{b}")
                    l8part = p1.tile([1, NSC], F32, tag="l8part", bufs=2, name=f"l8p{b}")
                    for sc in range(NSC):
                        nc.scalar.activation(
                            p_exp[0:1, ts(sc, SCH)], ps_scs[sc][:],
                            mybir.ActivationFunctionType.Exp,
                            bias=nmx[:],
                            accum_out=l8part[0:1, sc : sc + 1],
                        )
                    nc.vector.reduce_sum(l8row[0:1, b : b + 1], l8part[:],
                                         axis=mybir.AxisListType.X)
                    part4 = p1.tile([128, HC, NSC], F32, tag="part4", bufs=2,
                                    name=f"part4_{b}")
                    for sc in range(NSC):
                        p_bc = p1ps.tile([128, SCH], F32, tag="p_bc", bufs=2,
                                         name=f"p_bc{b}_{sc}")
                        nc.tensor.matmul(
                            p_bc[:], ones1[:], p_exp[0:1, ts(sc, SCH)],
                            start=True, stop=True,
                        )
                        for hc in range(HC):
                            prod = p1.tile([128, SCH], F32, tag="prod", bufs=2,
                                           name=f"prod{b}_{hc}_{sc}")
                            # fused multiply + row-sum in one DVE pass:
                            # out = (enc * 1.0) * p ; accum_out = sum(out)
                            nc.vector.scalar_tensor_tensor(
                                out=prod[:],
                                in0=enc_b[hc][:, ts(sc, SCH)].bitcast(F32),
                                scalar=1.0,
                                in1=p_bc[:],
                                op0=mybir.AluOpType.mult,
                                op1=mybir.AluOpType.mult,
                                accum_out=part4[:, hc, sc : sc + 1],
                            )
                    for hc in range(HC):
                        nc.vector.reduce_sum(
                            ctxTr[:, hc, b : b + 1], part4[:, hc, :],
                            axis=mybir.AxisListType.X,
                        )

                for b in range(BC + 1):
                    if b < BC:
                        stage_a(b)
                    if b > 0:
                        stage_b(b - 1)

            # ---- GRU (natural [b, feature] layout; b on partitions 0..7) ----
            with (
                tc.tile_pool(name="gru", bufs=1) as gp,
                tc.tile_pool(name="gru_psum", bufs=1, space="PSUM") as gps,
            ):
                # normalize context by 1/sum(exp): rl broadcast via K=1 matmul
                rl = gp.tile([1, BC], F32)
                nc.vector.reciprocal(rl[:], l8row[:])
                rl_bc = gps.tile([128, BC], F32, tag="rl_bc")
                nc.tensor.matmul(rl_bc[:], ones1[:].bitcast(F32), rl[:],
                                 start=True, stop=True)
                for hc in range(HC):
                    nc.vector.tensor_tensor(
                        ctxTf[:, hc, :], ctxTr[:, hc, :], rl_bc[:],
                        mybir.AluOpType.mult,
                    )
                    nc.scalar.copy(ctxTs[:, hc, :], ctxTf[:, hc, :])

                ps_gi = gps.tile([BC, 3 * H], F32)
                ps_gh = gps.tile([BC, 3 * H], F32)
                for kc in range(XKC):
                    lhs = embT[:, kc, :] if kc < EC else ctxTs[:, kc - EC, :]
                    for j in range(3 * H // 512):
                        nc.tensor.matmul(
                            ps_gi[:, ts(j, 512)], lhs, wih_t[:, kc, ts(j, 512)],
                            start=(kc == 0), stop=False,
                        )
                for kc in range(HC):
                    for j in range(3 * H // 512):
                        nc.tensor.matmul(
                            ps_gh[:, ts(j, 512)], hidT[:, kc, :], whh_t[:, kc, ts(j, 512)],
                            start=(kc == 0), stop=False,
                        )
                # biases as K=1 rank-1 accumulation: gi += 1^T b_ih ; gh += 1^T b_hh
                for j in range(3 * H // 512):
                    nc.tensor.matmul(
                        ps_gi[:, ts(j, 512)],
                        ones1[0:1, 0:BC], bih[0:1, ts(j, 512)],
                        start=False, stop=True,
                    )
                    nc.tensor.matmul(
                        ps_gh[:, ts(j, 512)],
                        ones1[0:1, 0:BC], bhh[0:1, ts(j, 512)],
                        start=False, stop=True,
                    )

                # gates (biases already accumulated): r,z = sigmoid(gi + gh)
                gi_sb = gp.tile([BC, 3 * H], F32)
                nc.scalar.copy(gi_sb[:], ps_gi[:])
                g_rz = gp.tile([BC, 2 * H], F32)
                nc.vector.tensor_tensor(g_rz[:], gi_sb[:, : 2 * H], ps_gh[:, : 2 * H],
                                        mybir.AluOpType.add)
                r_g = gp.tile([BC, H], F32)
                nc.scalar.activation(r_g[:], g_rz[:, :H],
                                     mybir.ActivationFunctionType.Sigmoid)
                z_g = gp.tile([BC, H], F32)
                nc.scalar.activation(z_g[:], g_rz[:, H:],
                                     mybir.ActivationFunctionType.Sigmoid)
                # n = tanh(gi_n + r * gh_n)
                u1 = gp.tile([BC, H], F32)
                nc.vector.tensor_tensor(u1[:], r_g[:], ps_gh[:, 2 * H :],
                                        mybir.AluOpType.mult)
                nc.vector.tensor_tensor(u1[:], u1[:], gi_sb[:, 2 * H :],
                                        mybir.AluOpType.add)
                n_g = gp.tile([BC, H], F32)
                nc.scalar.activation(n_g[:], u1[:], mybir.ActivationFunctionType.Tanh)
                # h_new = n + z * (hid - n)
                hmn = gp.tile([BC, H], F32)
                nc.vector.tensor_tensor(hmn[:], hidN[:], n_g[:], mybir.AluOpType.subtract)
                nc.vector.tensor_tensor(hmn[:], hmn[:], z_g[:], mybir.AluOpType.mult)
                h_new = gp.tile([BC, H], F32)
                nc.vector.tensor_tensor(h_new[:], n_g[:], hmn[:], mybir.AluOpType.add)
                nc.sync.dma_start(d_hnew[:], h_new[:])

                # hstackT = [h_new^T ; ctx^T]  (f32r)
                for hc in range(HC):
                    ps_t = gps.tile([128, BC], F32, tag="ps_t", bufs=1, name=f"ps_t{hc}")
                    nc.tensor.transpose(ps_t[:], h_new[:, ts(hc, 128)], eye8[:])
                    nc.scalar.copy(hstackT[:, hc, :], ps_t[:])
                    nc.scalar.copy(hstackT[:, HC + hc, :], ctxTf[:, hc, :])

            if stop_after == 'gru':
                return nc

            # ---- AllGather hstackT across cores ----
            cc_in = dpool.tile([128, GC, BC], dt_mm)
            nc.sync.dma_start(cc_in[:], hstackT[:])
            gathered = dpool.tile([NCORES, 128, GC, BC], dt_mm, addr_space="Shared")
            nc.gpsimd.collective_compute(
                "AllGather",
                mybir.AluOpType.bypass,
                replica_groups=[list(range(NCORES))],
                ins=[cc_in.opt()],
                outs=[gathered.opt()],
            )
            nc.sync.dma_start(
                hsT_all[:], gathered[:].rearrange("n p c b -> p c n b")
            )

            if stop_after == 'cc1':
                nc.sync.dma_start(d_out[0:64, 0:GC * BC], hsT_all[0:64, :, 0, :])
                return nc

            # ---- phase 2: logits for the local vocab slice + log_softmax ----
            with (
                tc.tile_pool(name="p2", bufs=1) as p2,
                tc.tile_pool(name="p2_psum", bufs=1, space="PSUM") as p2ps,
            ):
                outb = p2.tile([1, VC], dt_mm)
                nc.sync.dma_start(outb[:], d_outb[:])
                logits = p2.tile([B, VC], F32)
                lmax8 = p2.tile([B, 8], F32)
                lsum8 = p2.tile([B, 8], F32)
                ps_l = [p2ps.tile([B, VCH], F32, name=f"ps_l{j}") for j in range(8)]
                for kc in range(GC):
                    w_t = p2.tile([128, VC], dt_mm, tag="outw", bufs=2, name=f"outw{kc}")
                    nc.sync.dma_start(w_t[:], d_outWT[:, kc, :])
                    for j in range(8):
                        nc.tensor.matmul(
                            ps_l[j][:],
                            hsT_all[:, kc, :, :],
                            w_t[:, ts(j, VCH)],
                            start=(kc == 0),
                            stop=False,
                        )
                for j in range(8):
                    # out_b as K=1 rank-1 accumulation, then copy + row-max
                    nc.tensor.matmul(
                        ps_l[j][:],
                        ones1[0:1, 0:B], outb[0:1, ts(j, VCH)],
                        start=False, stop=True,
                    )
                    nc.scalar.copy(logits[:, ts(j, VCH)], ps_l[j][:])
                    nc.vector.reduce_max(lmax8[:, j : j + 1], logits[:, ts(j, VCH)],
                                         axis=mybir.AxisListType.X)
                lmax = p2.tile([B, 1], F32)
                nc.vector.reduce_max(lmax[:], lmax8[:], axis=mybir.AxisListType.X)
                nlmax = p2.tile([B, 1], F32)
                nc.scalar.mul(nlmax[:], lmax[:], -1.0)
                for j in range(8):
                    e_scr = p2.tile([B, VCH], F32, tag="escr", bufs=2, name=f"escr{j}")
                    nc.scalar.activation(
                        e_scr[:], logits[:, ts(j, VCH)],
                        mybir.ActivationFunctionType.Exp,
                        bias=nlmax[:],
                        accum_out=lsum8[:, j : j + 1],
                    )
                lsum = p2.tile([B, 1], F32)
                nc.vector.reduce_sum(lsum[:], lsum8[:], axis=mybir.AxisListType.X)

                # AllGather (lmax, lsum) pairs and combine
                st2 = p2.tile([B, 2], F32)
                nc.vector.tensor_copy(st2[:, 0:1], lmax[:])
                nc.vector.tensor_copy(st2[:, 1:2], lsum[:])
                cc2 = dpool.tile([B, 2], F32)
                nc.sync.dma_start(cc2[:], st2[:])
                g2d = dpool.tile([NCORES, B, 2], F32, addr_space="Shared")
                nc.gpsimd.collective_compute(
                    "AllGather",
                    mybir.AluOpType.bypass,
                    replica_groups=[list(range(NCORES))],
                    ins=[cc2.opt()],
                    outs=[g2d.opt()],
                )
                g2 = p2.tile([B, NCORES, 2], F32)
                nc.sync.dma_start(g2[:], g2d[:].rearrange("n p t -> p n t"))
                gmax = p2.tile([B, 1], F32)
                nc.vector.reduce_max(gmax[:], g2[:, :, 0], axis=mybir.AxisListType.X)
                ngmax = p2.tile([B, 1], F32)
                nc.scalar.mul(ngmax[:], gmax[:], -1.0)
                wexp = p2.tile([B, NCORES], F32)
                nc.scalar.activation(
                    wexp[:], g2[:, :, 0],
                    mybir.ActivationFunctionType.Exp,
                    bias=ngmax[:],
                )
                wsc = p2.tile([B, NCORES], F32)
                gsum = p2.tile([B, 1], F32)
                nc.vector.tensor_tensor(wsc[:], wexp[:], g2[:, :, 1],
                                        mybir.AluOpType.mult)
                nc.vector.reduce_sum(gsum[:], wsc[:], axis=mybir.AxisListType.X)
                lng = p2.tile([B, 1], F32)
                nc.scalar.activation(lng[:], gsum[:], mybir.ActivationFunctionType.Ln)
                corr = p2.tile([B, 1], F32)
                nc.vector.tensor_tensor(corr[:], gmax[:], lng[:], mybir.AluOpType.add)
                ncorr = p2.tile([B, 1], F32)
                nc.scalar.mul(ncorr[:], corr[:], -1.0)
                nc.vector.tensor_scalar_add(logits[:], logits[:], ncorr[:])
                nc.sync.dma_start(d_out[:], logits[:])

    return nc


# ------------------------------------------------------------------
# Host side
# ------------------------------------------------------------------

def tf32_round(x):
    """Round fp32 to TF32 (10-bit mantissa, RNE) — the PE's fp32r operand format."""
    u = np.ascontiguousarray(x, np.float32).view(np.uint32)
    r = (u + 0x0FFF + ((u >> 13) & 1)) & 0xFFFFE000
    return r.view(np.float32)


def _pack_chunks(mat):
    """[R, C] with R % 128 == 0 -> [128, R//128, C] chunk-major packing."""
    R, C = mat.shape
    return np.ascontiguousarray(mat.reshape(R // 128, 128, C).transpose(1, 0, 2))


def prepare_inputs(word_input, last_hidden, encoder_outputs, emb, attn_W, attn_b, v,
                   W_ih, W_hh, b_ih, b_hh, out_W, out_b):
    f = np.float32
    word = np.asarray(word_input).astype(np.int64)
    hid = np.asarray(last_hidden, f)[0]             # [B, H]
    enc = np.asarray(encoder_outputs, f)            # [S, B, H]
    emb = np.asarray(emb, f)
    attn_W = np.asarray(attn_W, f)
    attn_b = np.asarray(attn_b, f)
    v = np.asarray(v, f)
    W_ih = np.asarray(W_ih, f)
    W_hh = np.asarray(W_hh, f)
    b_ih = np.asarray(b_ih, f)
    b_hh = np.asarray(b_hh, f)
    out_W = np.asarray(out_W, f)
    out_b = np.asarray(out_b, f)
    S_ = enc.shape[0]
    V_ = out_W.shape[0]
    VC = V_ // NCORES

    # shared (replicated) tensors (pre-rounded to TF32 where they feed fp32r matmuls)
    attn_WTp = tf32_round(_pack_chunks(np.ascontiguousarray(attn_W.T)))  # [128, 8, 512]
    attn_bTp = np.ascontiguousarray(attn_b.reshape(4, 128).T)     # [128, 4]
    vTp = tf32_round(np.ascontiguousarray(v.reshape(4, 128).T))   # [128, 4]
    W_ihTp = tf32_round(_pack_chunks(np.ascontiguousarray(W_ih.T)))  # [128, 8, 1536]
    W_hhTp = tf32_round(_pack_chunks(np.ascontiguousarray(W_hh.T)))  # [128, 4, 1536]
    eye8 = np.eye(BC, dtype=f)
    ones1 = np.ones((1, 128), f)
    b_ihr = tf32_round(np.ascontiguousarray(b_ih.reshape(1, -1)))
    b_hhr = tf32_round(np.ascontiguousarray(b_hh.reshape(1, -1)))

    # enc transposed per batch: [B, 128, 4, S]
    encT = np.ascontiguousarray(enc.transpose(1, 2, 0))           # [B, H, S]
    encT = tf32_round(np.ascontiguousarray(
        encT.reshape(B, 4, 128, S_).transpose(0, 2, 1, 3)))       # [B, 128, 4, S]

    x_emb = emb[word]                                             # [B, E]
    out_WT = np.ascontiguousarray(out_W.T)                        # [2H, V]

    in_maps = []
    for c in range(NCORES):
        bs = slice(c * BC, (c + 1) * BC)
        vs = slice(c * VC, (c + 1) * VC)
        hid_c = np.ascontiguousarray(hid[bs])                     # [BC, H]
        hidTp = tf32_round(np.ascontiguousarray(
            hid_c.T.reshape(4, 128, BC).transpose(1, 0, 2)))      # [128, 4, BC]
        embTp = tf32_round(np.ascontiguousarray(
            x_emb[bs].T.reshape(4, 128, BC).transpose(1, 0, 2)))  # [128, 4, BC]
        out_WTp = tf32_round(_pack_chunks(np.ascontiguousarray(out_WT[:, vs])))  # [128, 8, VC]
        in_maps.append({
            "encTp": np.ascontiguousarray(encT[bs]),
            "attn_WTp": attn_WTp,
            "attn_bTp": attn_bTp,
            "vTp": vTp,
            "hidTp": hidTp,
            "embTp": embTp,
            "hid_nat": hid_c,
            "W_ihTp": W_ihTp,
            "W_hhTp": W_hhTp,
            "b_ihr": b_ihr,
            "b_hhr": b_hhr,
            "out_WTp": out_WTp,
            "out_bs": tf32_round(np.ascontiguousarray(out_b[vs].reshape(1, -1))),
            "eye8": eye8,
            "ones1": ones1,
        })
    return in_maps


_CACHE = {}


def get_nc(S_=S, VC=V // NCORES, stop_after='all'):
    key = (S_, VC, stop_after)
    if key not in _CACHE:
        nc = bacc.Bacc("TRN2", target_bir_lowering=False, debug=False,
                       enable_asserts=False, num_devices=NCORES)
        build(nc, S_=S_, VC=VC, stop_after=stop_after)
        nc.compile()
        _CACHE[key] = nc
    return _CACHE[key]


def run(in_maps, trace=False, stop_after='all', **kw):
    S_ = in_maps[0]["encTp"].shape[-1]
    VC = in_maps[0]["out_WTp"].shape[-1]
    nc = get_nc(S_, VC, stop_after)
    return run_bass_kernel_spmd(nc, in_maps, core_ids=list(range(NCORES)),
                                trace=trace, **kw)


def kernel(**inputs):
    in_maps = prepare_inputs(**inputs)
    res = run(in_maps)
    out = np.concatenate(
        [np.asarray(res.results[c]["out_logits"]) for c in range(NCORES)], axis=1)
    h_new = np.concatenate(
        [np.asarray(res.results[c]["h_new_out"]) for c in range(NCORES)], axis=0)
    return out, h_new[None]
